# revision 34
# baseline (speedup 1.0000x reference)
"""Trainium2 Bass kernel for nn_MemoryModule (retrieval_knn).

Reference computation (B=2, T=4, Ck=64, Cv=256, H=W=64, stride-2 maxpool):
  mk = maxpool(memory_keys)   -> [B,T,Ck,32,32] -> [B, M=4096, Ck]
  mv = maxpool(memory_values) -> [B,T,Cv,32,32] -> [B, Cv, M]
  attn = softmax_over_M(mk @ qk / sqrt(Ck))     # [B, M, N=4096]
  memory = mv @ attn                            # [B, Cv, N]
  out = concat([query_value, memory], ch axis)  # [B, 2*Cv, 64, 64]

Sharding over 8 cores: core c = 4*b + r handles batch b = c//4.
 - Loading/pooling is T-sharded: core loads memory_keys[b, r], memory_values[b, r],
   pools locally, then AllGathers the (small, bf16) pooled tensors within its
   4-core batch group.
 - Attention/softmax/PV is N-sharded: core handles query columns
   n in [1024*r, 1024*(r+1)). Softmax is over M which is fully local after the
   AllGather, so no distributed softmax is needed.
Matmuls run in bf16 (fp32 PSUM accumulation). Softmax skips max-subtraction
(logits ~ N(0, 1.25^2); exp is safe in fp32).
The softmax denominator comes for free as a 257th "ones" column appended to the
transposed pooled values: PV computes out^T[n, 0:256]=sum_m P*mv, out^T[n,256]=sum_m P.
"""

import sys

sys.path.insert(0, "/opt/trn_rl_repo")

import numpy as np

import concourse.bacc as bacc
import concourse.mybir as mybir
import concourse.tile as tile
from contextlib import ExitStack
from concourse.bass_utils import run_bass_kernel_spmd

N_CORES = 8
GROUPS = [[0, 1, 2, 3], [4, 5, 6, 7]]
F32 = mybir.dt.float32
BF16 = mybir.dt.bfloat16
EXP = mybir.ActivationFunctionType.Exp
BYPASS = mybir.AluOpType.bypass

_CACHE = {}


def _pool2x2(nc, out_ap, mid_ap, in_ap, h, w):
    """stride-2 2x2 maxpool along the free dims (h, w) -> (h/2, w/2)."""
    raw4 = in_ap.rearrange("c (h w2 two) -> c h w2 two", w2=w // 2, two=2)
    nc.vector.tensor_max(
        mid_ap.rearrange("c (h w one) -> c h w one", h=h, one=1),
        raw4[:, :, :, 0:1], raw4[:, :, :, 1:2])
    mid4 = mid_ap.rearrange("c (hp two w) -> c hp w two", hp=h // 2, two=2)
    nc.vector.tensor_max(
        out_ap.rearrange("c (h w one) -> c h w one", h=h // 2, one=1),
        mid4[:, :, :, 0:1], mid4[:, :, :, 1:2])


def _emit(nc, tc, io, use_collectives=True):
    """Emit the per-core program. io: dict of DRAM APs."""
    mk, mv, qk, qv = io["mk"], io["mv"], io["qk"], io["qv"]
    qv_out, memT_out = io["qv_out"], io["memT_out"]

    with ExitStack() as ctx:
        dram = ctx.enter_context(tc.tile_pool(name="dram", bufs=1, space="DRAM"))
        sb = ctx.enter_context(tc.tile_pool(name="persist", bufs=1))
        wk = ctx.enter_context(tc.tile_pool(name="work", bufs=2))
        sps = ctx.enter_context(tc.tile_pool(name="spsum", bufs=2, space="PSUM"))
        aps = ctx.enter_context(tc.tile_pool(name="apsum", bufs=4, space="PSUM"))
        pmat_pool = ctx.enter_context(tc.tile_pool(name="pmat", bufs=16))

        # ---- critical-path loads first: query key, then raw keys ----
        qkf = sb.tile([64, 1024], F32, name="qkf")
        nc.scalar.dma_start(qkf[:], qk[:])
        kraw = sb.tile([64, 4096], F32, name="kraw")
        nc.scalar.dma_start(kraw[:], mk[:])
        # raw values m-half-0 chunks on the ACT hwdge queue (no deps -> do
        # not block exps later); m-half-1 chunks are issued after the key
        # chain so keys win the DMA bus early.
        vraw = [sb.tile([128, 4096], F32, name=f"vraw{j}") for j in range(2)]
        for j in range(2):
            nc.scalar.dma_start(
                vraw[j][:, 0:2048], mv[128 * j:128 * (j + 1), 0:2048])

        # DVE: cast qk to bf16 first (QK-matmul critical path)
        qkb = sb.tile([64, 1024], BF16, name="qkb")
        nc.vector.tensor_copy(qkb[:], qkf[:])

        # ---- keys: pool, AllGather ----
        kpw = sb.tile([64, 2048], F32, name="kpw")
        kp = sb.tile([64, 1024], BF16, name="kp")
        _pool2x2(nc, kp[:], kpw[:], kraw[:], 64, 64)

        if use_collectives:
            kp_dram = dram.tile([64, 1024], BF16)
            kpg_dram = dram.tile([256, 1024], BF16)
            nc.sync.dma_start(kp_dram[:], kp[:])
            nc.gpsimd.collective_compute(
                "AllGather", BYPASS, replica_groups=GROUPS,
                ins=[kp_dram.opt()], outs=[kpg_dram.opt()])
            kpg = kpg_dram[:]
        else:
            kp_dram = dram.tile([64, 1024], BF16)
            nc.sync.dma_start(kp_dram[:], kp[:])
            kpg = io["kpg_in"]
        # [ck=64, m=4096] with m = t*1024 + local_m
        mkp = sb.tile([64, 4096], BF16, name="mkp")
        nc.sync.dma_start(
            mkp[:].rearrange("c (t m) -> c t m", t=4),
            kpg.rearrange("(t c) m -> c t m", c=64))

        # raw values m-half-1 chunks via gpsimd SWDGE; the WAR on vraw
        # (m-half-0 pooling reads) naturally delays them off the bus head
        for j in range(2):
            nc.gpsimd.dma_start(
                vraw[j][:, 2048:4096], mv[128 * j:128 * (j + 1), 2048:4096])

        # ---- values: pool + transpose per (cv-half, m-half) quarter,
        # ---- then one AllGather per m-half (keeps 512B DMA rows)
        # mvt{A,B} layout [m-part=128, blk, cv=257]; col 256 = ones.
        # AG output m-order: (t, m-half, local block) -> global m-tile
        # i = 8*t + 4*mh + blk, so mvt_of(i) = (A if (i%8)<4 else B,
        # 4*(i//8) + i%4).
        mvts = []
        for mh in range(2):
            vt = sb.tile([128, 4 * 256], BF16, name=f"vt{mh}")
            vt3 = vt[:].rearrange("p (i c) -> p i c", i=4)
            for j in range(2):
                vpw = sb.tile([128, 1024], F32, name=f"vpw{j}_{mh}")
                vpj = sb.tile([128, 512], BF16, name=f"vp{j}_{mh}")
                _pool2x2(nc, vpj[:], vpw[:],
                         vraw[j][:, 2048 * mh:2048 * (mh + 1)], 32, 64)
                # [128, 512] -> 3D out [128 m-part, blk=4, 128]
                nc.sync.dma_start_transpose(
                    vt3[:, :, 128 * j:128 * (j + 1)], vpj[:])
            if use_collectives:
                vt_dram = dram.tile([512, 256], BF16, name=f"vt_dram{mh}")
                vtg_dram = dram.tile([2048, 256], BF16, name=f"vtg_dram{mh}")
                nc.sync.dma_start(
                    vt_dram[:].rearrange("(i p) c -> p i c", p=128), vt3)
                nc.gpsimd.collective_compute(
                    "AllGather", BYPASS, replica_groups=GROUPS,
                    ins=[vt_dram.opt()], outs=[vtg_dram.opt()])
                vtg = vtg_dram[:]
            else:
                vt_dram = dram.tile([512, 256], BF16, name=f"vt_dram{mh}")
                nc.sync.dma_start(
                    vt_dram[:].rearrange("(i p) c -> p i c", p=128), vt3)
                vtg = io[f"vtg_in{mh}"]
            mvt = sb.tile([128, 16 * 257], BF16, name=f"mvt{mh}")
            mvt3 = mvt[:].rearrange("p (i c) -> p i c", i=16)
            nc.sync.dma_start(
                mvt3[:, :, 0:256],
                vtg.rearrange("(i p) c -> p i c", p=128))
            nc.vector.memset(mvt3[:, :, 256:257], 1.0)
            mvts.append(mvt3)

        def mvt_of(i):
            mh = (i % 8) // 4
            blk = 4 * (i // 8) + (i % 4)
            return mvts[mh][:, blk, :]

        # ---------------- query_value passthrough ----------------
        nc.sync.dma_start(qv_out[:], qv[:])

        # ------------- fused QK^T -> exp -> PV pipeline -------------
        # P[m, n] = exp(0.125 * sum_c mkp[c, m] * qk[c, n])
        # out^T[n, cv_aug] = sum_m P[m, n] * mvt[m, cv_aug]
        # N processed in two 512-column halves so 2x2-bank S-tiles +
        # 4 acc-banks fit in PSUM. m-tiles processed in pairs: two QK
        # matmuls fill a 2-bank S tile, one 1024-wide exp, 8 PV matmuls.
        # A 1-deep software pipeline overlaps exp(p) on ACT with PV(p-1)
        # on PE; deep pmat buffering lets the QK+exp front-end run ahead
        # while the values AllGather completes.
        # m-tile pairs ordered A-half-first: mvtA (m-half-0 of every t)
        # arrives before mvtB, so PV work exists as soon as AG2a lands.
        pair_ms = [8 * t + 4 * mh + 2 * u
                   for mh in range(2) for t in range(4) for u in range(2)]
        first_i = pair_ms[0]
        last_i = pair_ms[-1] + 1  # last m-tile index actually processed

        for half in range(2):
            accs = [aps.tile([128, 257], F32, name=f"acc{half}_{k}", tag="acc")
                    for k in range(4)]
            ptiles = {}
            qslice = slice(512 * half, 512 * (half + 1))

            def qk_exp(p):
                m0 = pair_ms[p]
                s_ps = sps.tile([128, 1024], F32, name="s_ps")
                for u in range(2):
                    nc.tensor.matmul(
                        s_ps[:, 512 * u:512 * (u + 1)],
                        mkp[:, 128 * (m0 + u):128 * (m0 + u + 1)],
                        qkb[:, qslice],
                        start=True, stop=True)
                pt = pmat_pool.tile([128, 1024], BF16, name="ptile")
                nc.scalar.activation(pt[:], s_ps[:], EXP, scale=0.125)
                ptiles[p] = pt

            def pv(p):
                pt = ptiles.pop(p)
                m0 = pair_ms[p]
                for u in range(2):
                    i = m0 + u
                    for k in range(4):
                        nc.tensor.matmul(
                            accs[k][:],
                            pt[:, 512 * u + 128 * k:512 * u + 128 * (k + 1)],
                            mvt_of(i),
                            start=(i == first_i), stop=(i == last_i))

            for p in range(17):
                if p < 16:
                    qk_exp(p)
                if p >= 1:
                    pv(p - 1)

            for k in range(4):
                kg = 4 * half + k
                acc = accs[k]
                rec = wk.tile([128, 1], F32, name="rec")
                nc.vector.reciprocal(rec[:], acc[:, 256:257])
                mo = wk.tile([128, 256], F32, name="mo")
                nc.vector.tensor_scalar_mul(mo[:], acc[:, 0:256], rec[:])
                nc.sync.dma_start(memT_out[128 * kg:128 * (kg + 1), :], mo[:])


def build(use_collectives=True):
    nc = bacc.Bacc("TRN2", target_bir_lowering=False, debug=False,
                   num_devices=N_CORES)
    io = {
        "mk": nc.dram_tensor("mk", [64, 4096], F32, kind="ExternalInput").ap(),
        "mv": nc.dram_tensor("mv", [256, 4096], F32, kind="ExternalInput").ap(),
        "qk": nc.dram_tensor("qk", [64, 1024], F32, kind="ExternalInput").ap(),
        "qv": nc.dram_tensor("qv", [256, 1024], F32, kind="ExternalInput").ap(),
        "qv_out": nc.dram_tensor("qv_out", [256, 1024], F32,
                                 kind="ExternalOutput").ap(),
        "memT_out": nc.dram_tensor("memT_out", [1024, 256], F32,
                                   kind="ExternalOutput").ap(),
    }
    if not use_collectives:
        io["kpg_in"] = nc.dram_tensor("kpg_in", [256, 1024], BF16,
                                      kind="ExternalInput").ap()
        io["vtg_in0"] = nc.dram_tensor("vtg_in0", [2048, 256], BF16,
                                       kind="ExternalInput").ap()
        io["vtg_in1"] = nc.dram_tensor("vtg_in1", [2048, 256], BF16,
                                       kind="ExternalInput").ap()
    with tile.TileContext(nc) as tc:
        _emit(nc, tc, io, use_collectives=use_collectives)
    nc.compile()
    return nc


def _get_nc():
    if "nc" not in _CACHE:
        _CACHE["nc"] = build(use_collectives=True)
    return _CACHE["nc"]


def make_in_maps(memory_keys, memory_values, query_key, query_value):
    B, T, Ck, H, W = memory_keys.shape
    Cv = memory_values.shape[2]
    N = H * W
    NL = N // 4
    mkf = np.ascontiguousarray(memory_keys.reshape(B, T, Ck, N), np.float32)
    mvf = np.ascontiguousarray(memory_values.reshape(B, T, Cv, N), np.float32)
    qkf = np.ascontiguousarray(query_key.reshape(B, Ck, N), np.float32)
    qvf = np.ascontiguousarray(query_value.reshape(B, Cv, N), np.float32)
    in_maps = []
    for c in range(N_CORES):
        b, r = divmod(c, 4)
        in_maps.append({
            "mk": np.ascontiguousarray(mkf[b, r]),
            "mv": np.ascontiguousarray(mvf[b, r]),
            "qk": np.ascontiguousarray(qkf[b, :, NL * r:NL * (r + 1)]),
            "qv": np.ascontiguousarray(qvf[b, :, NL * r:NL * (r + 1)]),
        })
    return in_maps


def assemble_output(results, B=2, Cv=256, H=64, W=64):
    N = H * W
    NL = N // 4
    out = np.empty((B, 2 * Cv, N), np.float32)
    for c in range(N_CORES):
        b, r = divmod(c, 4)
        sl = slice(NL * r, NL * (r + 1))
        out[b, :Cv, sl] = results[c]["qv_out"]
        out[b, Cv:, sl] = results[c]["memT_out"].T
    return out.reshape(B, 2 * Cv, H, W)


def kernel(memory_keys, memory_values, query_key, query_value, **_ignored):
    B, T, Ck, H, W = memory_keys.shape
    Cv = memory_values.shape[2]
    nc = _get_nc()
    in_maps = make_in_maps(memory_keys, memory_values, query_key, query_value)
    res = run_bass_kernel_spmd(nc, in_maps, core_ids=list(range(N_CORES)))
    return assemble_output(res.results, B=B, Cv=Cv, H=H, W=W)


if __name__ == "__main__":
    rng = np.random.default_rng(0)
    inputs = {
        "memory_keys": rng.standard_normal((2, 4, 64, 64, 64)).astype(np.float32),
        "memory_values": rng.standard_normal((2, 4, 256, 64, 64)).astype(np.float32),
        "query_key": rng.standard_normal((2, 64, 64, 64)).astype(np.float32),
        "query_value": rng.standard_normal((2, 256, 64, 64)).astype(np.float32),
    }
    out = kernel(**inputs)
    print("kernel output shape:", out.shape)
